# revision 53
# baseline (speedup 1.0000x reference)
"""Trainium2 Bass kernel for nn_CellularAutomatonDecoder.

Model (per reference):
  cells = embed[tokens] + pos_embed                        (B, T, D)
  rule_bias MLP from mean(c_states); const_bias = rule_bias @ W1b + b1
  8x CA steps: pre = cells@W1c + roll(cells,+1)@W1l + roll(cells,-1)@W1r + cb
               cells = a*cells + (1-a)*tanh(gelu(pre) @ W2 + b2)
  out = LN(cells) @ head_w                                 (B, T, V)

Sharding: pure data-parallel over batch across 8 cores (256 rows each).

Device design notes (v2 — fp8 DoubleRow + pipelined tail):
- feature-major state sigma[d=128, 8192] f32 in SBUF, t-major columns
  (col j = t*256 + b): the T-roll is a +-256 column shift.
- step modes "fffffttb": steps 0-4 full-fp8 (taps W1l/W1r and W2 as
  e4m3 DoubleRow pairs, center tap f32r), steps 5-6 tap-only-fp8
  (W2/gelu output stay f32), step 7 all-bf16 taps. Early steps carry the
  least error amplification, so fp8 goes there. PE per step drops from
  ~27.3us (f32r) to ~18.4us (full fp8) / ~21.4us (tap-fp8).
- fp8 shadow sig8 = 16*sigma with a 256-col halo on both sides so a
  DoubleRow moving operand is a plain contiguous slice:
  sig8[:, c0:c0+1024].rearrange("p (two n) -> p two n") pairs
  (sigma[j-256], sigma[j+256]) for output cols j in [c0, c0+512).
- scales: f32r/bf16-packed W1c x 32768, fp8 weights x 2048, sig8/h8 x 16
  -> every accumulating matmul lands in PSUM at x32768; descale via the
  activation scale argument (gelu scale=ia/32768, tanh scale=1/32768).
- token gather: bf16 tokens broadcast via K=1 ones-matmul (no 4MB DMA,
  warms the PE/HAM from ~1us), one-hot compares on DVE, embed via bf16
  one-hot matmul, pos added by the ACT copy out of PSUM.
- blends lag the chunk matmuls by 2 across ALL steps (no step barrier).
- tail: step 7 runs on bf16 taps; the final blend writes bf16 sigma
  directly; LN sums come token-major from N=1 ones-matmuls that reuse
  the head's sigma-block LDWEIGHTS (no PE micro-transposes, no [1,N]
  row copies); per-chunk inv-std math feeds the head scale; output DMAs
  start per 4-block group. PE stays dense to the end so HAM never
  re-throttles (the old kernel ran its last 50us at half clock).
"""

import os
import sys

import numpy as np

for _p in ("/opt/trn_rl_repo", "/root/.axon_site/_ro/trn_rl_repo"):
    if os.path.isdir(_p) and _p not in sys.path:
        sys.path.append(_p)

from contextlib import ExitStack

import ml_dtypes

import concourse.bacc as bacc
import concourse.tile as tile
from concourse import mybir
from concourse.bass_utils import run_bass_kernel_spmd

F32 = mybir.dt.float32
F32R = mybir.dt.float32r
BF16 = mybir.dt.bfloat16
FP8 = mybir.dt.float8e4
AF = mybir.ActivationFunctionType
ALU = mybir.AluOpType
AX = mybir.AxisListType
DR = mybir.MatmulPerfMode.DoubleRow

B, T, D, V, CDIM = 2048, 32, 128, 256, 128
NEV = 8
EPS = 1e-5
NC = 8
BL = B // NC          # 256 batch rows per core
NTOK = BL * T         # 8192 tokens per core
CH = 1024             # token chunk (columns)
NCH = NTOK // CH      # 8 chunks
NBLK = NTOK // 128    # 64 head blocks

SC = 2048.0           # f32r center-tap / fp8 weight / PSUM scale
SX = 16.0             # fp8 h-activation extra scale
MODES = "fffffttb"    # per-step: f=full fp8, t=tap-only fp8, b=bf16

TRACE = False
_CACHE = {}


def _pieces(dst0, n, shift, maxlen=512):
    """Contiguous (dst, src, len) pieces of src = (dst + shift) mod NTOK."""
    out = []
    j = 0
    while j < n:
        s = (dst0 + j + shift) % NTOK
        ln = min(n - j, NTOK - s, maxlen)
        out.append((dst0 + j, s, ln))
        j += ln
    return out


def _build(a, has_lnb):
    ia = 1.0 - a
    nc = bacc.Bacc("TRN2", target_bir_lowering=False, debug=False, num_devices=NC)

    tok_d = nc.dram_tensor("tok", [1, NTOK], BF16, kind="ExternalInput").ap()
    ones_d = nc.dram_tensor("onesb", [128, 4], BF16, kind="ExternalInput").ap()
    cpack_d = nc.dram_tensor("cpack", [128, 44], F32, kind="ExternalInput").ap()
    epack_d = nc.dram_tensor("epack", [128, 256], BF16, kind="ExternalInput").ap()
    wca_d = nc.dram_tensor("wca", [128, 256], F32R, kind="ExternalInput").ap()
    w2f_d = nc.dram_tensor("w2f", [128, 256], F32R, kind="ExternalInput").ap()
    wb_d = nc.dram_tensor("wb", [128, 1280], BF16, kind="ExternalInput").ap()
    w8_d = nc.dram_tensor("w8", [128, 768], FP8, kind="ExternalInput").ap()
    fpack_d = nc.dram_tensor("fpack", [128, 768], F32, kind="ExternalInput").ap()
    out_d = nc.dram_tensor("out", [NTOK, V], F32, kind="ExternalOutput").ap()
    out_r = out_d.rearrange("(b t) v -> b t v", t=T)

    with tile.TileContext(nc) as tc, ExitStack() as ctx:
        # ---- persistent SBUF ----
        # DMA priority: tokens/cpack/epack feed init; w8/wca feed the first
        # fp8 steps; wb (step 7 weights) can land last.
        wpool = ctx.enter_context(tc.tile_pool(name="weights", bufs=1))
        tokbc = wpool.tile([128, NTOK], BF16, tag="tokbc")

        def tokdma(eng, g):
            src = tok_d[0:1, g * CH:(g + 1) * CH].broadcast_to((128, CH))
            eng.dma_start(tokbc[:, g * CH:(g + 1) * CH], src)

        epack = wpool.tile([128, 256], BF16, tag="epack")
        cpack = wpool.tile([128, 44], F32, tag="cpack")
        onesb = wpool.tile([128, 4], BF16, tag="onesb")
        vidb = onesb[:, 2:4]
        wca = wpool.tile([128, 256], F32R, tag="wca")
        w2f = wpool.tile([128, 256], F32R, tag="w2f")
        wb = wpool.tile([128, 1280], BF16, tag="wb")
        w8 = wpool.tile([128, 768], FP8, tag="w8")
        fpack = wpool.tile([128, 768], F32, tag="fpack")

        nc.scalar.dma_start(cpack[:], cpack_d)
        tokdma(nc.sync, 0)
        nc.scalar.dma_start(fpack[:], fpack_d)
        tokdma(nc.sync, 1)
        nc.scalar.dma_start(epack[:], epack_d)
        tokdma(nc.sync, 2)
        tokdma(nc.scalar, 3)
        nc.sync.dma_start(w8[:], w8_d)
        tokdma(nc.scalar, 4)
        tokdma(nc.sync, 5)
        nc.scalar.dma_start(wca[:], wca_d)
        tokdma(nc.sync, 6)
        tokdma(nc.scalar, 7)
        nc.sync.dma_start(w2f[:], w2f_d)
        nc.scalar.dma_start(onesb[:], ones_d)
        nc.sync.dma_start(wb[:], wb_d)

        posT_s, cT_s = cpack[:, 0:32], cpack[:, 32:36]
        bc1_s, bc2_s = cpack[:, 36:38], cpack[:, 38:39]
        b1_s, b2_s = cpack[:, 39:41], cpack[:, 41:42]
        vid_s = cpack[:, 42:44]
        wcb, wlb, wrb = wb[:, 0:256], wb[:, 256:512], wb[:, 512:768]
        w2b, hwcb = wb[:, 768:1024], wb[:, 1024:1280]
        w1b_s, wc1_s, wc2_s = fpack[:, 0:256], fpack[:, 256:512], fpack[:, 512:768]

        spool = ctx.enter_context(tc.tile_pool(name="state", bufs=1))
        sig = spool.tile([128, NTOK], F32R, tag="sigma")
        sig8 = spool.tile([128, NTOK + 512], FP8, tag="sig8")
        sigb = spool.tile([128, NTOK], BF16, tag="sigb")

        mlp_sb = ctx.enter_context(tc.tile_pool(name="mlp_sb", bufs=1))
        cbias_s = mlp_sb.tile([128, 2], F32, tag="cbias")

        # shared pools, all phases (no release barriers)
        pp = ctx.enter_context(tc.tile_pool(name="psum", bufs=1, space="PSUM"))
        sbhb = ctx.enter_context(tc.tile_pool(name="hb_sb", bufs=2))
        sbh8 = ctx.enter_context(tc.tile_pool(name="h8_sb", bufs=2))
        sbhf = ctx.enter_context(tc.tile_pool(name="hf_sb", bufs=4))
        sbt = ctx.enter_context(tc.tile_pool(name="t_sb", bufs=4))
        sbsq = ctx.enter_context(tc.tile_pool(name="sq_sb", bufs=2))
        sbst = ctx.enter_context(tc.tile_pool(name="stat_sb", bufs=3))
        sbo = ctx.enter_context(tc.tile_pool(name="out_sb", bufs=3))

        def pre_t(name, cols=CH):
            return pp.tile([128, cols], F32, tag="pre", name=name, bufs=3)

        def new_t(shape, name):
            return pp.tile(shape, F32, tag="new", name=name, bufs=1)

        # ---- PE warmup: ~5us of dense full-K matmuls so HAM unthrottles
        # before the real init stream. Source is a memset tile, not a DMA'd
        # one, so the dummies start during the input DMAs (~6us earlier).
        wdum = wpool.tile([128, 256], BF16, tag="wdum")
        nc.gpsimd.memset(wdum[:], 1.0)
        warm_ps = new_t([128, 512], "warm_ps")
        for wi in range(40):
            nc.tensor.matmul(warm_ps[:, 0:256], wdum[:, 0:128], wdum[:],
                             start=True, stop=True)

        # ---- rule-bias MLP (tiny; first in program order, overlaps DMAs) ----
        cp_s = mlp_sb.tile([128, 1], F32, tag="cp")
        nc.vector.tensor_reduce(cp_s[:], cT_s[:], axis=AX.X, op=ALU.add)
        y1_ps = new_t([128, 2], "y1_ps")
        for h in range(2):
            nc.tensor.matmul(y1_ps[:, h:h + 1], wc1_s[:, h * 128:(h + 1) * 128],
                             cp_s[:], start=True, stop=True)
        y1g_s = mlp_sb.tile([128, 2], F32, tag="y1g")
        for h in range(2):
            nc.scalar.activation(y1g_s[:, h:h + 1], y1_ps[:, h:h + 1], AF.Gelu,
                                 bias=bc1_s[:, h:h + 1], scale=0.25)
        rb_ps = new_t([128, 2], "rb_ps")
        nc.tensor.matmul(rb_ps[:, 0:1], wc2_s[:, 0:128], y1g_s[:, 0:1],
                         start=True, stop=False)
        nc.tensor.matmul(rb_ps[:, 0:1], wc2_s[:, 128:256], y1g_s[:, 1:2],
                         start=False, stop=True)
        rb_s = mlp_sb.tile([128, 1], F32, tag="rb")
        nc.scalar.activation(rb_s[:], rb_ps[:, 0:1], AF.Identity, bias=bc2_s[:, 0:1])
        cb_ps = new_t([128, 2], "cb_ps")
        for h in range(2):
            nc.tensor.matmul(cb_ps[:, h:h + 1], w1b_s[:, h * 128:(h + 1) * 128],
                             rb_s[:], start=True, stop=True)
        for h in range(2):
            nc.scalar.activation(cbias_s[:, h:h + 1], cb_ps[:, h:h + 1], AF.Identity,
                                 bias=b1_s[:, h:h + 1])

        # ---- init: one-hot on DVE (4x bf16 tier), embed matmul ----
        for ci in range(NCH):
            c0 = ci * CH
            oh = sbhb.tile([128, 2 * CH], BF16, tag="hb", name="oh")
            nc.vector.tensor_scalar(oh[:, 0:CH], tokbc[:, c0:c0 + CH],
                                    vid_s[:, 0:1], None, ALU.is_equal)
            nc.vector.tensor_scalar(oh[:, CH:2 * CH], tokbc[:, c0:c0 + CH],
                                    vid_s[:, 1:2], None, ALU.is_equal)
            cells_ps = pre_t("cells_ps")
            for k in range(2):
                jc = slice(k * 512, (k + 1) * 512)
                nc.tensor.matmul(cells_ps[:, jc], epack[:, 0:128],
                                 oh[:, k * 512:(k + 1) * 512],
                                 start=True, stop=False)
                nc.tensor.matmul(cells_ps[:, jc], epack[:, 128:256],
                                 oh[:, CH + k * 512:CH + (k + 1) * 512],
                                 start=False, stop=True)
            for kb in range(CH // 256):
                tt = (c0 + kb * 256) // 256
                nc.scalar.activation(sig[:, c0 + kb * 256: c0 + (kb + 1) * 256],
                                     cells_ps[:, kb * 256:(kb + 1) * 256],
                                     AF.Identity, bias=posT_s[:, tt:tt + 1])
            nc.vector.tensor_copy(sig8[:, 256 + c0:256 + c0 + CH],
                                   sig[:, c0:c0 + CH])
            if ci == NCH - 1:
                nc.vector.tensor_copy(sig8[:, 0:256], sig[:, NTOK - 256:NTOK])
            if ci == 0:
                nc.vector.tensor_copy(sig8[:, 256 + NTOK:512 + NTOK],
                                      sig[:, 0:256])

        # ---- evolve ----
        w8r = w8[:].rearrange("p (x m) -> p x m", m=128)  # x: wl0,wr0,wl1,wr1,w20,w21
        sig8r = sig8  # halo offset: sigma col c maps to sig8 col c+256

        def emit_fp8_pre(ci):
            c0 = ci * CH
            pre = [pre_t(f"pre{h}") for h in range(2)]
            for h in range(2):
                hcols = slice(h * 128, (h + 1) * 128)
                lhs8 = w8[:, h * 256:(h + 1) * 256].rearrange(
                    "p (two m) -> p two m", two=2)
                for k in range(2):
                    c0k = c0 + k * 512
                    jc = slice(k * 512, (k + 1) * 512)
                    nc.tensor.matmul(pre[h][:, jc], wca[:, hcols],
                                     sig[:, c0k:c0k + 512], start=True, stop=False)
                    rhs8 = sig8r[:, c0k:c0k + 1024].rearrange(
                        "p (two n) -> p two n", two=2)
                    nc.tensor.matmul(pre[h][:, jc], lhs8, rhs8,
                                     start=False, stop=True, perf_mode=DR)
            return pre

        def emit_stage1(ci, mode):
            """pre matmuls + gelu (+ fp8 cast of h). Returns h for stage 2."""
            c0 = ci * CH
            if mode in ("f", "t"):
                pre = emit_fp8_pre(ci)
            else:  # 'b': bf16 taps from sigb
                pre = [pre_t(f"pre{h}") for h in range(2)]
                for h in range(2):
                    hcols = slice(h * 128, (h + 1) * 128)
                    for k in range(2):
                        jc = slice(k * 512, (k + 1) * 512)
                        nc.tensor.matmul(pre[h][:, jc], wcb[:, hcols],
                                         sigb[:, c0 + k * 512:c0 + (k + 1) * 512],
                                         start=True, stop=False)
                    for dd, ss, ll in _pieces(c0, CH, -256):
                        nc.tensor.matmul(pre[h][:, dd - c0:dd - c0 + ll],
                                         wlb[:, hcols], sigb[:, ss:ss + ll],
                                         start=False, stop=False)
                    for dd, ss, ll in _pieces(c0, CH, +256):
                        nc.tensor.matmul(pre[h][:, dd - c0:dd - c0 + ll],
                                         wrb[:, hcols], sigb[:, ss:ss + ll],
                                         start=False, stop=True)
            if mode == "f":
                h_b = sbhb.tile([128, 2 * CH], BF16, tag="hb", name="h_b")
                for h in range(2):
                    nc.scalar.activation(h_b[:, h * CH:(h + 1) * CH], pre[h][:],
                                         AF.Gelu, bias=cbias_s[:, h:h + 1],
                                         scale=ia / SC)
                h_8 = sbh8.tile([128, 2 * CH], FP8, tag="h8", name="h_8")
                for h in range(2):
                    nc.vector.tensor_scalar(h_8[:, h * CH:(h + 1) * CH],
                                            h_b[:, h * CH:(h + 1) * CH],
                                            SX, None, ALU.mult)
                return h_8
            elif mode == "t":
                h_f = [sbhf.tile([128, CH], F32R, tag="hf", name=f"hf{h}")
                       for h in range(2)]
                for h in range(2):
                    nc.scalar.activation(h_f[h][:], pre[h][:], AF.Gelu,
                                         bias=cbias_s[:, h:h + 1], scale=ia / SC)
                return h_f
            else:
                h_b = sbhb.tile([128, 2 * CH], BF16, tag="hb", name="h_b")
                for h in range(2):
                    nc.scalar.activation(h_b[:, h * CH:(h + 1) * CH], pre[h][:],
                                         AF.Gelu, bias=cbias_s[:, h:h + 1],
                                         scale=ia)
                return h_b

        def emit_stage2(ci, mode, hin):
            """W2 matmuls + tanh -> t tile."""
            new_ps = new_t([128, CH], "new_ps")
            if mode == "f":
                lhsw2 = w8[:, 512:768].rearrange("p (two m) -> p two m", two=2)
                h8r = hin[:].rearrange("p (two n) -> p two n", two=2)
                for k in range(2):
                    jc = slice(k * 512, (k + 1) * 512)
                    nc.tensor.matmul(new_ps[:, jc], lhsw2, h8r[:, :, jc],
                                     start=True, stop=True, perf_mode=DR)
                tanh_scale = 1.0 / (SC * SX)
            elif mode == "t":
                for k in range(2):
                    jc = slice(k * 512, (k + 1) * 512)
                    nc.tensor.matmul(new_ps[:, jc], w2f[:, 0:128], hin[0][:, jc],
                                     start=True, stop=False)
                    nc.tensor.matmul(new_ps[:, jc], w2f[:, 128:256], hin[1][:, jc],
                                     start=False, stop=True)
                tanh_scale = 1.0
            else:
                for k in range(2):
                    jc = slice(k * 512, (k + 1) * 512)
                    nc.tensor.matmul(new_ps[:, jc], w2b[:, 0:128],
                                     hin[:, k * 512:(k + 1) * 512],
                                     start=True, stop=False)
                    nc.tensor.matmul(new_ps[:, jc], w2b[:, 128:256],
                                     hin[:, CH + k * 512:CH + (k + 1) * 512],
                                     start=False, stop=True)
                tanh_scale = 1.0
            t_t = sbt.tile([128, CH], F32, tag="t", name="t_t")
            nc.scalar.activation(t_t[:], new_ps[:], AF.Tanh, bias=b2_s[:, 0:1],
                                 scale=tanh_scale)
            return t_t

        def emit_blend(s, ci, t_t):
            c0 = ci * CH
            if s == NEV - 1:
                # final blend: write bf16 state for stats + head
                nc.vector.scalar_tensor_tensor(
                    sigb[:, c0:c0 + CH], sig[:, c0:c0 + CH], a, t_t[:],
                    op0=ALU.mult, op1=ALU.add)
                return
            nc.vector.scalar_tensor_tensor(
                sig[:, c0:c0 + CH], sig[:, c0:c0 + CH], a, t_t[:],
                op0=ALU.mult, op1=ALU.add)
            if s <= 5:
                nc.vector.tensor_copy(sig8[:, 256 + c0:256 + c0 + CH],
                                      sig[:, c0:c0 + CH])
                if ci == NCH - 1:
                    nc.vector.tensor_copy(sig8[:, 0:256],
                                          sig[:, NTOK - 256:NTOK])
                if ci == 0:
                    nc.vector.tensor_copy(sig8[:, 256 + NTOK:512 + NTOK],
                                          sig[:, 0:256])
            if s == 6:
                nc.vector.tensor_copy(sigb[:, c0:c0 + CH], sig[:, c0:c0 + CH])

        def emit_tail(ci):
            # stats + head for chunk ci (final sigma in sigb)
            c0 = ci * CH
            # dependency-free LDWEIGHTS filler: keeps the PE array active for
            # HAM during the tail's cross-engine waits (no PSUM, no deps), so
            # the clock gate never drops to K=4 while stats/head work drains
            for _ in range(14):
                nc.tensor.ldweights(wdum[:, 0:128])
            sq = sbsq.tile([128, CH], BF16, tag="sq", name="sq")
            nc.scalar.activation(sq[:], sigb[:, c0:c0 + CH], AF.Square)
            # merged head+stats sweep: each sigma-block is loaded as PE
            # weights once, serving the head matmul (N=256) and the s1 sum
            # (N=1) back-to-back; the sq-block load follows for s2. Head
            # results drain to SBUF unscaled right away (PSUM freed fast).
            st_ps = new_t([128, 16], "st_ps")
            t0 = 4 * ci
            o_ts = []
            for hh in range(2):
                o_t = sbo.tile([128, CH], F32, tag="o", name="o_t")
                aps = [pre_t(f"a{hh}{i}", cols=512) for i in range(2)]
                for tl in range(4):
                    ap_ = aps[tl // 2]
                    blk = 2 * (t0 + tl) + hh
                    bc = blk * 128
                    j = hh * 4 + tl
                    nc.tensor.matmul(ap_[:, (tl % 2) * 256:(tl % 2 + 1) * 256],
                                     sigb[:, bc:bc + 128], hwcb[:],
                                     start=True, stop=True)
                    nc.tensor.matmul(st_ps[:, 2 * j:2 * j + 1],
                                     sigb[:, bc:bc + 128], onesb[:, 0:1],
                                     start=True, stop=True)
                    nc.tensor.matmul(st_ps[:, 2 * j + 1:2 * j + 2],
                                     sq[:, bc - c0:bc - c0 + 128], onesb[:, 0:1],
                                     start=True, stop=True)
                    if tl % 2 == 1:
                        nc.vector.tensor_copy(
                            o_t[:, (tl - 1) * V:(tl + 1) * V], ap_[:])
                o_ts.append(o_t)
            # inv-std on DVE (no ACT Sqrt -> no table switches):
            # quake rsqrt seed + 2 Newton iterations
            g = nc.vector
            st3 = st_ps[:].rearrange("p (b two) -> p b two", two=2)
            s1ap, s2ap = st3[:, :, 0], st3[:, :, 1]
            m1 = sbst.tile([128, 8], F32, tag="m2", name="m1")
            g.tensor_scalar(m1[:], s1ap, ia / 128.0, None, ALU.mult)
            m2 = sbst.tile([128, 8], F32, tag="m2", name="m2")
            g.tensor_mul(m2[:], m1[:], m1[:])
            vf = sbst.tile([128, 8], F32, tag="vf", name="vf")
            g.scalar_tensor_tensor(vf[:], s2ap, ia * ia / 128.0, m2[:],
                                   op0=ALU.mult, op1=ALU.subtract)
            g.tensor_scalar_add(vf[:], vf[:], EPS)
            us = sbst.tile([128, 8], F32, tag="m2", name="us")
            g.tensor_scalar(us[:].bitcast(mybir.dt.uint32),
                            vf[:].bitcast(mybir.dt.uint32),
                            1, None, ALU.logical_shift_right)
            usf = sbst.tile([128, 8], F32, tag="m2", name="usf")
            g.tensor_copy(usf[:], us[:].bitcast(mybir.dt.uint32))
            yf = sbst.tile([128, 8], F32, tag="m2", name="yf")
            g.tensor_scalar(yf[:], usf[:], -1.0, float(0x5F375A86),
                            ALU.mult, op1=ALU.add)
            y0 = sbst.tile([128, 8], F32, tag="m2", name="y0")
            g.tensor_copy(y0[:].bitcast(mybir.dt.uint32), yf[:])
            yy = y0
            for it in range(1):
                r = sbst.tile([128, 8], F32, tag="vf", name=f"r{it}")
                g.tensor_mul(r[:], vf[:], yy[:])
                g.tensor_mul(r[:], r[:], yy[:])
                w_ = sbst.tile([128, 8], F32, tag="vf", name=f"w{it}")
                g.tensor_scalar(w_[:], r[:], -0.5, 1.5, ALU.mult, op1=ALU.add)
                yn = sbst.tile([128, 8], F32, tag="inv" if it == 0 else "m2",
                               name=f"y{it + 1}")
                g.tensor_mul(yn[:], yy[:], w_[:])
                yy = yn
            inv = yy
            for hh in range(2):
                o_t = o_ts[hh]
                for tl in range(4):
                    j = hh * 4 + tl
                    nc.vector.tensor_scalar(o_t[:, tl * V:(tl + 1) * V],
                                            o_t[:, tl * V:(tl + 1) * V],
                                            inv[:, j:j + 1], None, ALU.mult)
                nc.sync.dma_start(out_r[hh * 128:hh * 128 + 128, t0:t0 + 4, :],
                                  o_t[:].rearrange("p (t v) -> p t v", t=4))

        # software-pipelined emission: stage1(c) | stage2(c-1) | blend(c-2),
        # flowing across step boundaries with no barrier. stage2 of chunk c
        # runs one chunk behind its stage1 so the in-order ACT queue never
        # blocks a ready gelu behind a not-yet-ready tanh.
        work = []   # (step, chunk) in emission order
        for s in range(NEV):
            work += [(s, (s + 1 + j) % NCH) for j in range(NCH)]
        hbuf, tbuf = {}, {}
        for idx, (s, ci) in enumerate(work):
            hbuf[(s, ci)] = emit_stage1(ci, MODES[s])
            if idx >= 1:
                s1, c1 = work[idx - 1]
                tbuf[(s1, c1)] = emit_stage2(c1, MODES[s1], hbuf.pop((s1, c1)))
            if idx >= 2:
                s2, c2 = work[idx - 2]
                emit_blend(s2, c2, tbuf.pop((s2, c2)))
                if s2 == NEV - 1:
                    emit_tail(c2)
        for idx in (len(work) - 1,):
            s1, c1 = work[idx]
            tbuf[(s1, c1)] = emit_stage2(c1, MODES[s1], hbuf.pop((s1, c1)))
        for idx in (len(work) - 2, len(work) - 1):
            s2, c2 = work[idx]
            emit_blend(s2, c2, tbuf.pop((s2, c2)))
            emit_tail(c2)

    nc.compile()
    return nc


def kernel(**inputs):
    g = {k: np.asarray(v, np.float32) if k != "tokens" else np.asarray(v)
         for k, v in inputs.items()}
    alpha = float(g["alpha"])
    a = float(1.0 / (1.0 + np.exp(-np.float64(alpha))))
    ia = np.float32(1.0 - a)
    ln_b = g["ln_b"]
    has_lnb = bool(np.any(ln_b != 0))
    key = (np.float64(a).tobytes(), has_lnb)
    if key not in _CACHE:
        _CACHE[key] = _build(a, has_lnb)
    nc = _CACHE[key]

    W1, W2 = g["W1"], g["W2"]
    W1c, W1l, W1r, W1b = W1[:D], W1[D:2 * D], W1[2 * D:3 * D], W1[3 * D:]
    embed, pos = g["embed"], g["pos_embed"]
    head_w, ln_g = g["head_w"], g["ln_g"]

    bf = ml_dtypes.bfloat16
    e4 = ml_dtypes.float8_e4m3

    onesb = np.ones((128, 4), np.float32)
    onesb[:, 2] = np.arange(128)
    onesb[:, 3] = np.arange(128, 256)
    onesb = onesb.astype(bf)

    cpack = np.zeros((128, 44), np.float32)
    cpack[:, 0:32] = pos.T * np.float32(1.0 / ia)
    cpack[:, 32:36] = g["c_states"].T
    cpack[:, 36:38] = g["bc1"].reshape(2, 128).T
    cpack[:, 38:39] = g["bc2"].reshape(128, 1)
    cpack[:, 39:41] = g["b1"].reshape(2, 128).T
    cpack[:, 41:42] = g["b2"].reshape(128, 1)
    cpack[:, 42:44] = np.stack([np.arange(128), np.arange(128, 256)], axis=1)

    epack = (np.concatenate([embed[0:128], embed[128:256]], axis=1)
             * np.float32(1.0 / ia)).astype(bf)

    wca = (W1c * np.float32(SC)).astype(np.float32)
    w2f = np.concatenate([W2[0:128], W2[128:256]], axis=1).astype(np.float32)

    ghw = head_w * ln_g[:, None]
    hwc = (ghw - ghw.mean(axis=0, keepdims=True)) * ia
    wb = np.zeros((128, 1280), np.float32)
    wb[:, 0:256] = W1c
    wb[:, 256:512] = W1l
    wb[:, 512:768] = W1r
    wb[:, 768:1024] = np.concatenate([W2[0:128], W2[128:256]], axis=1)
    wb[:, 1024:1280] = hwc
    wb = wb.astype(bf)

    w8 = np.zeros((128, 768), np.float32)
    w8[:, 0:128] = W1l[:, 0:128]
    w8[:, 128:256] = W1r[:, 0:128]
    w8[:, 256:384] = W1l[:, 128:256]
    w8[:, 384:512] = W1r[:, 128:256]
    w8[:, 512:640] = W2[0:128]
    w8[:, 640:768] = W2[128:256]
    w8 = np.clip(w8 * np.float32(SC), -240, 240).astype(e4)

    fpack = np.zeros((128, 768), np.float32)
    fpack[:, 0:256] = W1b
    fpack[:, 256:512] = g["Wc1"]
    fpack[:, 512:768] = np.concatenate([g["Wc2"][0:128], g["Wc2"][128:256]], axis=1)

    tokens = g["tokens"]
    in_maps = []
    for c in range(NC):
        tk = tokens[c * BL:(c + 1) * BL].astype(np.float32)   # (BL, T)
        in_maps.append({
            "tok": np.ascontiguousarray(tk.T).reshape(1, NTOK).astype(bf),
            "onesb": onesb, "cpack": cpack, "epack": epack,
            "wca": wca, "w2f": w2f, "wb": wb, "w8": w8, "fpack": fpack,
        })

    kw = {}
    if TRACE:
        kw = dict(trace=True)
    res = run_bass_kernel_spmd(nc, in_maps, core_ids=list(range(NC)), **kw)
    if TRACE and res.exec_time_ns is not None:
        print(f"HW exec time: {res.exec_time_ns} ns")
        kernel.last_exec_ns = res.exec_time_ns
        kernel.last_trace = res.instructions_and_trace
    out = np.stack([res.results[c]["out"] for c in range(NC)], axis=0)
    out = out.reshape(B, T, V)
    if has_lnb:
        out = out + (ln_b @ head_w)[None, None, :]
    return np.ascontiguousarray(out)
